# revision 1
# baseline (speedup 1.0000x reference)
"""Trainium2 Bass kernel for nn_Decoder: 1024-step GRU decoder, H=2048, OD=512.

Strategy (8 NeuronCores, tensor-parallel):
  - Host: merge r/z gates of W_ih+W_hh (valid since x==h after step 0), collapse
    the activation-free 5-linear chain into one [512,2048] matrix, run step 0 on
    host (x=0 there, so the merge doesn't apply).
  - Device: 1023 strictly-sequential GRU steps. Gate GEMVs row-sharded 8 ways
    (1024 rows/core/step); weights live in SBUF (bf16) for the whole kernel.
    Per-step h exchange via remote_dma_broadcast (SBUF->SBUF, all 8 cores,
    landing slot indexed by sender id register) — no collective_compute.
  - h history accumulates in SBUF; after the loop one batched GEMM computes
    all 1024 outputs against the collapsed chain matrix.
"""

import os
import sys

import numpy as np
import ml_dtypes

for p in ("/opt/trn_rl_repo", "/root/.axon_site/_ro/trn_rl_repo"):
    if os.path.isdir(p) and p not in sys.path:
        sys.path.append(p)

import concourse.bass as bass
import concourse.mybir as mybir
from concourse import library_config

H = 2048
OD = 512
T = 1024
N_CORES = 8
NK = 16  # contraction chunks of 128
NR = 8   # gate row tiles of 128 per core
ROWS = NR * 128  # 1024 gate rows per core
OC = OD // N_CORES  # 64 output cols per core

BF16 = mybir.dt.bfloat16
F32 = mybir.dt.float32

LAST_RESULTS = None  # stash of the most recent BassKernelResults (for test.py)


def build_nc(t_hist=T):
    """Build the SPMD program. t_hist = number of h-history slots (incl. slot 0);
    device runs S = t_hist-1 recurrence steps."""
    S = t_hist - 1
    af = mybir.ActivationFunctionType

    def rcnt(tau):
        # number of steps <= tau with the same parity as tau
        return tau // 2 + (tau % 2)

    nc = bass.Bass("TRN2", target_bir_lowering=False, debug=False,
                   num_devices=N_CORES)

    # DRAM I/O (per-core data; same shapes on every core)
    wg_d = nc.declare_dram_parameter("wg", [128, NK * NR * 128], BF16, isOutput=False)
    wc_d = nc.declare_dram_parameter("wc", [128, NK * OC], BF16, isOutput=False)
    bg_d = nc.declare_dram_parameter("bg", [128, NR], F32, isOutput=False)
    bc_d = nc.declare_dram_parameter("bc", [OC, 1], F32, isOutput=False)
    h1b_d = nc.declare_dram_parameter("h1b", [128, NK], BF16, isOutput=False)
    h1f_d = nc.declare_dram_parameter("h1f", [128, 2], F32, isOutput=False)
    out_d = nc.declare_dram_parameter("out", [OC, t_hist], F32, isOutput=True)
    cc_in = [nc.dram_tensor(f"cc_in{i}", [2 * 128], BF16) for i in range(2)]
    cc_out = [nc.dram_tensor(f"cc_out{i}", [NK * 128], BF16, addr_space="Shared") for i in range(2)]

    n_gemm_tiles = (t_hist + 511) // 512

    from contextlib import ExitStack
    with ExitStack() as ctx:
        E = ctx.enter_context
        s_wg = E(nc.sbuf_tensor("s_wg", [128, NK * NR * 128], BF16))
        s_wc = E(nc.sbuf_tensor("s_wc", [128, NK * OC], BF16))
        s_bg = E(nc.sbuf_tensor("s_bg", [128, NR], F32))
        s_bc = E(nc.sbuf_tensor("s_bc", [OC, 1], F32))
        s_hist = E(nc.sbuf_tensor("s_hist", [128, NK * t_hist], BF16))
        s_g = E(nc.sbuf_tensor("s_g", [128, NR], F32))
        s_rz = E(nc.sbuf_tensor("s_rz", [128, 4], F32))
        s_t1 = E(nc.sbuf_tensor("s_t1", [128, 2], F32))
        s_t2 = E(nc.sbuf_tensor("s_t2", [128, 2], F32))
        s_n = E(nc.sbuf_tensor("s_n", [128, 2], F32))
        s_d = E(nc.sbuf_tensor("s_d", [128, 2], F32))
        s_zd = E(nc.sbuf_tensor("s_zd", [128, 2], F32))
        s_hf = E(nc.sbuf_tensor("s_hf", [128, 2], F32))
        s_hbf = E(nc.sbuf_tensor("s_hbf", [128, 2], BF16))
        s_ob = [E(nc.sbuf_tensor(f"s_ob{i}", [OC, 512], F32)) for i in range(n_gemm_tiles)]

        pg = [E(nc.psum_tensor(f"pg{i}", [128, 512], F32)) for i in range(2)]
        po = [E(nc.psum_tensor(f"po{i}", [128, 512], F32)) for i in range(n_gemm_tiles)]

        dma_in = E(nc.semaphore("dma_in"))
        dma_out = E(nc.semaphore("dma_out"))
        s_pe = E(nc.semaphore("s_pe"))
        s1 = E(nc.semaphore("s1"))
        s2 = E(nc.semaphore("s2"))
        s3 = E(nc.semaphore("s3"))
        s4 = E(nc.semaphore("s4"))
        s5 = E(nc.semaphore("s5"))
        s6 = E(nc.semaphore("s6"))
        in_sem = E(nc.semaphore("in_sem"))
        cc_sem = E(nc.semaphore("cc_sem"))
        hsem = E(nc.semaphore("hsem"))
        s_gemm = E(nc.semaphore("s_gemm"))
        s_out = E(nc.semaphore("s_out"))
        d1 = E(nc.semaphore("d1"))
        d2 = E(nc.semaphore("d2"))
        d3 = E(nc.semaphore("d3"))

        with nc.Block() as block:

            @block.sync
            def _(sp):
                sp.dma_start(out=s_wg[:, :], in_=wg_d[:, :]).then_inc(dma_in, 16)
                sp.dma_start(out=s_wc[:, :], in_=wc_d[:, :]).then_inc(dma_in, 16)
                sp.dma_start(out=s_bg[:, :], in_=bg_d[:, :]).then_inc(dma_in, 16)
                sp.dma_start(out=s_bc[:, :], in_=bc_d[:, :]).then_inc(dma_in, 16)
                sp.dma_start(out=s_hist[:, 0:NK], in_=h1b_d[:, :]).then_inc(dma_in, 16)
                sp.dma_start(out=s_hf[:, :], in_=h1f_d[:, :]).then_inc(dma_in, 16)
                for t in range(1, S + 1):
                    b = t % 2
                    sp.wait_ge(s6, t)
                    if t >= 3:
                        sp.wait_ge(cc_sem, t - 2)  # WAR: cc_in[b] read by CC(t-2)
                    sp.dma_start(out=cc_in[b][:], in_=s_hbf[:, :]).then_inc(in_sem, 16)
                    sp.wait_ge(cc_sem, t)
                    sp.dma_start(
                        out=s_hist[:, t * NK : (t + 1) * NK],
                        in_=cc_out[b].ap().rearrange("(p f) -> p f", f=NK),
                    ).then_inc(hsem, 16)
                for tt in range(n_gemm_tiles):
                    ntt = min(512, t_hist - tt * 512)
                    sp.wait_ge(s_out, tt + 1)
                    sp.dma_start(
                        out=out_d[0:OC, tt * 512 : tt * 512 + ntt],
                        in_=s_ob[tt][0:OC, 0:ntt],
                    ).then_inc(dma_out, 16)
                sp.wait_ge(dma_out, 16 * n_gemm_tiles)

            @block.tensor
            def _(te):
                te.wait_ge(dma_in, 96)
                for t in range(1, S + 1):
                    if t >= 2:
                        te.wait_ge(hsem, 16 * (t - 1))
                    p = pg[t % 2]
                    for rt in range(NR):
                        for c in range(NK):
                            mm = te.matmul(
                                p[:, rt : rt + 1],
                                lhsT=s_wg[:, (c * NR + rt) * 128 : (c * NR + rt + 1) * 128],
                                rhs=s_hist[:, (t - 1) * NK + c : (t - 1) * NK + c + 1],
                                start=(c == 0),
                                stop=(c == NK - 1),
                            )
                    mm.then_inc(s_pe, 1)
                # final GEMM: out[m, tau] = sum_k Wc[m,k] hist[tau,k]
                if S >= 1:
                    te.wait_ge(hsem, 16 * S)
                for tt in range(n_gemm_tiles):
                    ntt = min(512, t_hist - tt * 512)
                    for c in range(NK):
                        mm = te.matmul(
                            po[tt][0:OC, 0:ntt],
                            lhsT=s_wc[:, c * OC : (c + 1) * OC],
                            rhs=bass.AP(s_hist, tt * 512 * NK + c, [[NK * t_hist, 128], [NK, ntt]]),
                            start=(c == 0),
                            stop=(c == NK - 1),
                        )
                    mm.then_inc(s_gemm, 1)

            @block.vector
            def _(ve):
                ve.wait_ge(dma_in, 96)
                for t in range(1, S + 1):
                    p = pg[t % 2]
                    ve.wait_ge(s_pe, t)
                    if t >= 2:
                        # WAR: s_g/s_t1 read by op3/op4 of step t-1
                        ve.wait_ge(d1, t - 1)
                        ve.wait_ge(s4, t - 1)
                    ve.tensor_add(s_g[:, :], p[:, 0:NR], s_bg[:, :]).then_inc(s1, 1)
                    ve.wait_ge(s2, t)
                    ve.tensor_mul(s_t1[:, :], s_rz[:, 0:2], s_g[:, 6:8]).then_inc(d1, 1)
                    ve.wait_ge(d1, t)
                    ve.tensor_add(s_t2[:, :], s_t1[:, :], s_g[:, 4:6]).then_inc(s4, 1)
                    ve.wait_ge(s3, t)
                    if t >= 2:
                        ve.wait_ge(s5, t - 1)  # RAW: s_hf written by op8 of step t-1
                    ve.tensor_sub(s_d[:, :], s_hf[:, :], s_n[:, :]).then_inc(d2, 1)
                    ve.wait_ge(d2, t)
                    ve.tensor_mul(s_zd[:, :], s_d[:, :], s_rz[:, 2:4]).then_inc(d3, 1)
                    ve.wait_ge(d3, t)
                    ve.tensor_add(s_hf[:, :], s_n[:, :], s_zd[:, :]).then_inc(s5, 1)

            @block.scalar
            def _(se):
                se.wait_ge(dma_in, 96)
                for t in range(1, S + 1):
                    se.wait_ge(s1, t)
                    if t >= 2:
                        se.wait_ge(d3, t - 1)  # WAR: s_rz read by op7 of step t-1
                    se.activation(s_rz[:, :], s_g[:, 0:4], af.Sigmoid).then_inc(s2, 1)
                    se.wait_ge(s4, t)
                    if t >= 2:
                        se.wait_ge(s5, t - 1)  # WAR: s_n read by op8 of step t-1
                    se.activation(s_n[:, :], s_t2[:, :], af.Tanh).then_inc(s3, 1)
                    se.wait_ge(s5, t)
                    if t >= 2:
                        se.wait_ge(in_sem, 16 * (t - 1))
                    se.activation(s_hbf[:, :], s_hf[:, :], af.Copy).then_inc(s6, 1)
                for tt in range(n_gemm_tiles):
                    ntt = min(512, t_hist - tt * 512)
                    se.wait_ge(s_gemm, tt + 1)
                    se.activation(
                        s_ob[tt][0:OC, 0:ntt], po[tt][0:OC, 0:ntt],
                        af.Identity, bias=s_bc[0:OC, 0:1],
                    ).then_inc(s_out, 1)

            @block.gpsimd
            def _(g):
                for t in range(1, S + 1):
                    b = t % 2
                    g.wait_ge(in_sem, 16 * t)
                    if t >= 3:
                        g.wait_ge(hsem, 16 * (t - 2))  # WAR: cc_out[b] read by out-dma(t-2)
                    g.collective_compute(
                        "AllGather",
                        mybir.AluOpType.bypass,
                        replica_groups=[list(range(N_CORES))],
                        ins=[cc_in[b][:]],
                        outs=[cc_out[b][:]],
                    ).then_inc(cc_sem, 1)

    return nc


def _host_precompute(inputs):
    """fp64 host precompute: merged gates, collapsed chain, step 0."""
    f64 = lambda k: np.asarray(inputs[k], np.float64)
    W_ih, W_hh = f64("W_ih"), f64("W_hh")
    b_ih, b_hh = f64("b_ih"), f64("b_hh")

    W_rz = W_ih[: 2 * H] + W_hh[: 2 * H]
    b_rz = b_ih[: 2 * H] + b_hh[: 2 * H]
    W_in_, b_in_ = W_ih[2 * H :], b_ih[2 * H :]
    W_hn, b_hn = W_hh[2 * H :], b_hh[2 * H :]

    W_chain = f64("Wo") @ f64("W4") @ f64("W3") @ f64("W2") @ f64("W1")
    b_chain = (
        f64("Wo") @ (f64("W4") @ (f64("W3") @ (f64("W2") @ f64("b1") + f64("b2")) + f64("b3")) + f64("b4"))
        + f64("bo")
    )

    sig = lambda x: 1.0 / (1.0 + np.exp(-x))
    h0 = f64("hidden")[0]
    gi = b_ih
    gh = W_hh @ h0 + b_hh
    r = sig(gi[:H] + gh[:H])
    z = sig(gi[H : 2 * H] + gh[H : 2 * H])
    n = np.tanh(gi[2 * H :] + r * gh[2 * H :])
    h1 = ((1.0 - z) * n + z * h0).astype(np.float32)

    return W_rz, b_rz, W_in_, b_in_, W_hn, b_hn, W_chain, b_chain, h1


def _perm():
    # hist SBUF layout: hist[p, f] = h[PERM[p, f]], chosen so both exchange
    # DMAs are contiguous: gathered[gpos] with gpos = p*16+f, where
    # gpos = i*256 + p2*2 + c  <->  h[256*i + 128*c + p2]
    k = np.arange(128)[:, None]
    f = np.arange(NK)[None, :]
    return 256 * (k // NK) + 128 * (f % 2) + (k % NK) * 8 + f // 2


PERM = _perm()


def make_in_maps(inputs):
    W_rz, b_rz, W_in_, b_in_, W_hn, b_hn, W_chain, b_chain, h1 = _host_precompute(inputs)
    bf = lambda x: np.ascontiguousarray(x.astype(np.float32).astype(ml_dtypes.bfloat16))
    f32 = lambda x: np.ascontiguousarray(x.astype(np.float32))

    in_maps = []
    for i in range(N_CORES):
        sl = slice(256 * i, 256 * (i + 1))
        rows = np.concatenate([W_rz[:H][sl], W_rz[H:][sl], W_in_[sl], W_hn[sl]], axis=0)  # [1024, 2048]
        # wg[k, (f*NR+rt)*128+m] = rows[rt*128+m, PERM[k, f]]
        wg = rows.reshape(NR, 128, H)[:, :, PERM].transpose(2, 3, 0, 1).reshape(128, NK * NR * 128)
        wci = W_chain[OC * i : OC * (i + 1)]  # [64, 2048]
        wc = wci[:, PERM].transpose(1, 2, 0).reshape(128, NK * OC)
        bias_rows = np.concatenate([b_rz[:H][sl], b_rz[H:][sl], b_in_[sl], b_hn[sl]])  # [1024]
        bg = np.ascontiguousarray(bias_rows.reshape(NR, 128).T)  # [128, NR]
        bc = b_chain[OC * i : OC * (i + 1)].reshape(OC, 1)
        h1b = np.ascontiguousarray(h1[PERM])  # [128, NK]
        h1f = np.ascontiguousarray(h1[sl].reshape(2, 128).T)  # [128, 2]
        in_maps.append({
            "wg": bf(wg), "wc": bf(wc), "bg": f32(bg), "bc": f32(bc),
            "h1b": bf(h1b), "h1f": f32(h1f),
        })
    return in_maps


def kernel(**inputs):
    global LAST_RESULTS
    from concourse.bass_utils import run_bass_kernel_spmd

    in_maps = make_in_maps(inputs)
    nc = build_nc(T)
    res = run_bass_kernel_spmd(nc, in_maps, core_ids=list(range(N_CORES)))
    LAST_RESULTS = res

    out = np.empty((T, OD), np.float32)
    for i in range(N_CORES):
        out[:, OC * i : OC * (i + 1)] = res.results[i]["out"].T
    return out[:, None, :]



# revision 2
# speedup vs baseline: 3.4917x; 3.4917x over previous
"""Trainium2 Bass kernel for nn_Decoder: 1024-step GRU decoder, H=2048, OD=512.

Same math as baseline (host-merged r/z gates, collapsed 5-linear chain, step 0
on host, bf16 TP-8 gate GEMVs with weights resident in SBUF), but the per-step
h exchange uses remote_dma_broadcast (SBUF->SBUF, all 8 cores, one instruction
per step, landing slot indexed by a sender-id register) instead of
collective_compute AllGather + DRAM bounce DMAs.

Exchange lands in a small double-buffered pad s_land[128, 2*NK] (big-stride APs
don't encode in the remote-DMA descs); the DVE archives each slot into s_hist
off the critical path for the final output GEMM.

exchange modes:
  'rdma'  - remote-DMA exchange, baseline gate chain.
  'rdma3' - rdma + PE r/z-first split (gate activations overlap the in/hn
            matmuls), shortened post-tanh chain (zh = z*h and 1-z precomputed
            under the PE), bf16 h state (DVE writes the send buffer directly;
            no scalar-engine copy on the critical path).
  'none'  - no exchange at all (reads stale slot 0); timing skeleton only.
"""

import os
import sys

import numpy as np
import ml_dtypes

for p in ("/opt/trn_rl_repo", "/root/.axon_site/_ro/trn_rl_repo"):
    if os.path.isdir(p) and p not in sys.path:
        sys.path.append(p)

import concourse.bass as bass
import concourse.mybir as mybir
from concourse import library_config

H = 2048
OD = 512
T = 1024
N_CORES = 8
NK = 16  # contraction chunks of 128
NR = 8   # gate row tiles of 128 per core
ROWS = NR * 128  # 1024 gate rows per core
OC = OD // N_CORES  # 64 output cols per core

BF16 = mybir.dt.bfloat16
F32 = mybir.dt.float32

LAST_RESULTS = None


def build_nc(t_hist=T, exchange="rdma3"):
    """Build the SPMD program. t_hist = number of h-history slots (incl. slot 0);
    device runs S = t_hist-1 recurrence steps."""
    S = t_hist - 1
    af = mybir.ActivationFunctionType

    nc = bass.Bass("TRN2", target_bir_lowering=False, debug=False,
                   num_devices=N_CORES)

    wg_d = nc.declare_dram_parameter("wg", [128, NK * NR * 128], BF16, isOutput=False)
    wc_d = nc.declare_dram_parameter("wc", [128, NK * OC], BF16, isOutput=False)
    bg_d = nc.declare_dram_parameter("bg", [128, NR], F32, isOutput=False)
    bc_d = nc.declare_dram_parameter("bc", [OC, 1], F32, isOutput=False)
    h1b_d = nc.declare_dram_parameter("h1b", [128, NK], BF16, isOutput=False)
    h1f_d = nc.declare_dram_parameter("h1f", [128, 2], F32, isOutput=False)
    h1fb_d = nc.declare_dram_parameter("h1fb", [128, 2], BF16, isOutput=False)
    out_d = nc.declare_dram_parameter("out", [OC, t_hist], F32, isOutput=True)

    n_gemm_tiles = (t_hist + 511) // 512

    from contextlib import ExitStack
    with ExitStack() as ctx:
        E = ctx.enter_context
        s_wg = E(nc.sbuf_tensor("s_wg", [128, NK * NR * 128], BF16))
        s_wc = E(nc.sbuf_tensor("s_wc", [128, NK * OC], BF16))
        s_bg = E(nc.sbuf_tensor("s_bg", [128, NR], F32))
        s_bc = E(nc.sbuf_tensor("s_bc", [OC, 1], F32))
        s_hist = E(nc.sbuf_tensor("s_hist", [128, NK * t_hist], BF16))
        s_land = E(nc.sbuf_tensor("s_land", [128, 2 * NK], BF16))  # 2 parities
        s_g = E(nc.sbuf_tensor("s_g", [128, NR], F32))
        s_rz = E(nc.sbuf_tensor("s_rz", [128, 4], F32))
        s_t1 = E(nc.sbuf_tensor("s_t1", [128, 2], F32))
        s_t2 = E(nc.sbuf_tensor("s_t2", [128, 2], F32))
        s_n = E(nc.sbuf_tensor("s_n", [128, 2], F32))
        s_d = E(nc.sbuf_tensor("s_d", [128, 2], F32))
        s_zd = E(nc.sbuf_tensor("s_zd", [128, 2], F32))
        s_hf = E(nc.sbuf_tensor("s_hf", [128, 2], F32))
        s_hbf = E(nc.sbuf_tensor("s_hbf", [128, 4], BF16))  # send buf, 2 slots
        s_grz = E(nc.sbuf_tensor("s_grz", [128, 4], F32))
        s_gin = E(nc.sbuf_tensor("s_gin", [128, 4], F32))
        s_zh = E(nc.sbuf_tensor("s_zh", [128, 2], F32))
        s_omz = E(nc.sbuf_tensor("s_omz", [128, 2], F32))
        s_c = E(nc.sbuf_tensor("s_c", [128, 2], F32))
        s_ob = [E(nc.sbuf_tensor(f"s_ob{i}", [OC, 512], F32)) for i in range(n_gemm_tiles)]

        pg = [E(nc.psum_tensor(f"pg{i}", [128, 512], F32)) for i in range(2)]
        pgb = [E(nc.psum_tensor(f"pgb{i}", [128, 512], F32)) for i in range(2)]
        po = [E(nc.psum_tensor(f"po{i}", [128, 512], F32)) for i in range(n_gemm_tiles)]
        n_dummy = int(os.environ.get("N_DUMMY", "0"))

        dma_in = E(nc.semaphore("dma_in"))
        dma_out = E(nc.semaphore("dma_out"))
        s_pe = E(nc.semaphore("s_pe"))
        s_peb = E(nc.semaphore("s_peb"))
        s1 = E(nc.semaphore("s1"))
        s2 = E(nc.semaphore("s2"))
        s3 = E(nc.semaphore("s3"))
        s4 = E(nc.semaphore("s4"))
        s5 = E(nc.semaphore("s5"))
        s6 = E(nc.semaphore("s6"))
        s_gemm = E(nc.semaphore("s_gemm"))
        s_out = E(nc.semaphore("s_out"))
        d1 = E(nc.semaphore("d1"))
        d2 = E(nc.semaphore("d2"))
        d3 = E(nc.semaphore("d3"))
        rd = exchange in ("rdma", "rdma3")
        if rd:
            rsem = E(nc.semaphore("rsem"))   # +2 per arriving slice; 16/step
            lsem = E(nc.semaphore("lsem"))   # +16 per completed local send
            psem = E(nc.semaphore("psem"))   # +1 per desc-gen prep
            csem = E(nc.semaphore("csem"))   # +1 per archived hist slot

        NIN = 128  # 8 initial DMAs x 16

        with nc.Block() as block:

            @block.sync
            def _(sp):
                sp.dma_start(out=s_wg[:, :], in_=wg_d[:, :]).then_inc(dma_in, 16)
                sp.dma_start(out=s_wc[:, :], in_=wc_d[:, :]).then_inc(dma_in, 16)
                sp.dma_start(out=s_bg[:, :], in_=bg_d[:, :]).then_inc(dma_in, 16)
                sp.dma_start(out=s_bc[:, :], in_=bc_d[:, :]).then_inc(dma_in, 16)
                sp.dma_start(out=s_hist[:, 0:NK], in_=h1b_d[:, :]).then_inc(dma_in, 16)
                sp.dma_start(out=s_land[:, 0:NK], in_=h1b_d[:, :]).then_inc(dma_in, 16)
                sp.dma_start(out=s_hf[:, :], in_=h1f_d[:, :]).then_inc(dma_in, 16)
                sp.dma_start(out=s_hbf[:, 0:2], in_=h1fb_d[:, :]).then_inc(dma_in, 16)
                for tt in range(n_gemm_tiles):
                    ntt = min(512, t_hist - tt * 512)
                    sp.wait_ge(s_out, tt + 1)
                    sp.dma_start(
                        out=out_d[0:OC, tt * 512 : tt * 512 + ntt],
                        in_=s_ob[tt][0:OC, 0:ntt],
                    ).then_inc(dma_out, 16)
                sp.wait_ge(dma_out, 16 * n_gemm_tiles)

            @block.tensor
            def _(te):
                te.wait_ge(dma_in, NIN)
                for t in range(1, S + 1):
                    if rd:
                        if t >= 2:
                            te.wait_ge(rsem, 16 * (t - 1))
                        rhs_base = NK * ((t - 1) % 2)
                    else:
                        if t >= 2:
                            te.wait_ge(s6, t - 1)
                        rhs_base = 0
                    p = pg[t % 2]
                    if exchange == "rdma3":
                        # r/z tiles (rt 0-3) first so gate activations overlap
                        # the in/hn tiles. rt-outer (groups must not interleave
                        # within a bank); separate banks per group so the DVE
                        # can read group A while group B still accumulates.
                        pb = pgb[t % 2]
                        for rt in range(NR):
                            dst = p[:, rt : rt + 1] if rt < 4 else pb[:, rt - 4 : rt - 3]
                            for c in range(NK):
                                mm = te.matmul(
                                    dst,
                                    lhsT=s_wg[:, (c * NR + rt) * 128 : (c * NR + rt + 1) * 128],
                                    rhs=s_land[:, rhs_base + c : rhs_base + c + 1],
                                    start=(c == 0),
                                    stop=(c == NK - 1),
                                )
                            if rt == 3:
                                mm.then_inc(s_pe, 1)
                        mm.then_inc(s_peb, 1)
                        # keep the PE clocked up through the exchange window
                        # (po[0] gets start=True-reset by the final GEMM later)
                        for w in range(n_dummy if t < S else 0):
                            te.matmul(
                                po[0][:, 0:1],
                                lhsT=s_wg[:, (w % 128) * 128 : (w % 128) * 128 + 128],
                                rhs=s_land[:, rhs_base : rhs_base + 1],
                                start=True,
                                stop=True,
                            )
                    else:
                        for rt in range(NR):
                            for c in range(NK):
                                mm = te.matmul(
                                    p[:, rt : rt + 1],
                                    lhsT=s_wg[:, (c * NR + rt) * 128 : (c * NR + rt + 1) * 128],
                                    rhs=s_land[:, rhs_base + c : rhs_base + c + 1],
                                    start=(c == 0),
                                    stop=(c == NK - 1),
                                )
                        mm.then_inc(s_pe, 1)
                # final GEMM: out[m, tau] = sum_k Wc[m,k] hist[tau,k]
                if rd and S >= 1:
                    te.wait_ge(csem, S)  # all slots archived
                for tt in range(n_gemm_tiles):
                    ntt = min(512, t_hist - tt * 512)
                    for c in range(NK):
                        mm = te.matmul(
                            po[tt][0:OC, 0:ntt],
                            lhsT=s_wc[:, c * OC : (c + 1) * OC],
                            rhs=bass.AP(s_hist, tt * 512 * NK + c, [[NK * t_hist, 128], [NK, ntt]]),
                            start=(c == 0),
                            stop=(c == NK - 1),
                        )
                    mm.then_inc(s_gemm, 1)

            @block.vector
            def _(ve):
                ve.wait_ge(dma_in, NIN)
                for t in range(1, S + 1):
                    b = t % 2
                    # archive slot t-1 (parity (t-1)%2) into s_hist; off the
                    # critical path. Safe vs bcast t+1 (same parity): senders
                    # gate on our step-t gates, which follow this in VE order.
                    if rd and t >= 2:
                        ve.wait_ge(rsem, 16 * (t - 1))
                        ve.tensor_scalar_add(
                            s_hist[:, (t - 1) * NK : t * NK],
                            s_land[:, NK * ((t - 1) % 2) : NK * ((t - 1) % 2) + NK],
                            0.0,
                        ).then_inc(csem, 1)
                    p = pg[t % 2]
                    if exchange == "rdma3":
                        ve.wait_ge(s_pe, t)
                        if t >= 2:
                            ve.wait_ge(s2, t - 1)  # WAR s_grz (sigmoid read)
                        ve.tensor_add(s_grz[:, :], p[:, 0:4], s_bg[:, 0:4]).then_inc(s1, 1)
                        ve.wait_ge(s2, t)
                        if t >= 2:
                            ve.wait_ge(s5, t - 1)  # V8(t-1) retired: s_zh/s_omz WAR
                            ve.wait_ge(s6, t - 1)  # f32 master h updated (V9 t-1)
                        # under PE group B: omz = 1-z, zh = z*h (f32 master)
                        ve.tensor_scalar(
                            s_omz[:, :], s_rz[:, 2:4], -1.0, 1.0,
                            mybir.AluOpType.mult, mybir.AluOpType.add,
                        )
                        ve.tensor_mul(s_zh[:, :], s_rz[:, 2:4], s_hf[:, :])
                        ve.wait_ge(s_peb, t)
                        if t >= 2:
                            ve.wait_ge(s4, t - 1)  # WAR s_gin/s_t1 (V5/V6 t-1)
                        ve.tensor_add(s_gin[:, :], pgb[t % 2][:, 0:4], s_bg[:, 4:8]).then_inc(d1, 1)
                        ve.wait_ge(d1, t)
                        ve.tensor_mul(s_t1[:, :], s_rz[:, 0:2], s_gin[:, 2:4]).then_inc(d2, 1)
                        ve.wait_ge(d2, t)
                        if t >= 2:
                            ve.wait_ge(s3, t - 1)  # WAR s_t2 (tanh read)
                        ve.tensor_add(s_t2[:, :], s_t1[:, :], s_gin[:, 0:2]).then_inc(s4, 1)
                        ve.wait_ge(s3, t)
                        ve.tensor_mul(s_c[:, :], s_omz[:, :], s_n[:, :]).then_inc(d3, 1)
                        ve.wait_ge(d3, t)
                        if t >= 3:
                            ve.wait_ge(lsem, 16 * (t - 2))  # WAR send slot b
                        ve.tensor_add(s_hbf[:, 2 * b : 2 * b + 2], s_c[:, :], s_zh[:, :]).then_inc(s5, 1)
                        # f32 master h, off the critical path (read by V2 t+1)
                        ve.tensor_add(s_hf[:, :], s_c[:, :], s_zh[:, :]).then_inc(s6, 1)
                    else:
                        ve.wait_ge(s_pe, t)
                        if t >= 2:
                            ve.wait_ge(d1, t - 1)
                            ve.wait_ge(s4, t - 1)
                        ve.tensor_add(s_g[:, :], p[:, 0:NR], s_bg[:, :]).then_inc(s1, 1)
                        ve.wait_ge(s2, t)
                        ve.tensor_mul(s_t1[:, :], s_rz[:, 0:2], s_g[:, 6:8]).then_inc(d1, 1)
                        ve.wait_ge(d1, t)
                        ve.tensor_add(s_t2[:, :], s_t1[:, :], s_g[:, 4:6]).then_inc(s4, 1)
                        ve.wait_ge(s3, t)
                        if t >= 2:
                            ve.wait_ge(s5, t - 1)
                        ve.tensor_sub(s_d[:, :], s_hf[:, :], s_n[:, :]).then_inc(d2, 1)
                        ve.wait_ge(d2, t)
                        ve.tensor_mul(s_zd[:, :], s_d[:, :], s_rz[:, 2:4]).then_inc(d3, 1)
                        ve.wait_ge(d3, t)
                        ve.tensor_add(s_hf[:, :], s_n[:, :], s_zd[:, :]).then_inc(s5, 1)
                if rd and S >= 1:
                    ve.wait_ge(rsem, 16 * S)
                    ve.tensor_scalar_add(
                        s_hist[:, S * NK : (S + 1) * NK],
                        s_land[:, NK * (S % 2) : NK * (S % 2) + NK],
                        0.0,
                    ).then_inc(csem, 1)

            @block.scalar
            def _(se):
                se.wait_ge(dma_in, NIN)
                for t in range(1, S + 1):
                    b = t % 2
                    if exchange == "rdma3":
                        se.wait_ge(s1, t)
                        if t >= 2:
                            se.wait_ge(s4, t - 1)  # WAR s_rz (VE reads)
                        se.activation(s_rz[:, :], s_grz[:, :], af.Sigmoid).then_inc(s2, 1)
                        se.wait_ge(s4, t)
                        if t >= 2:
                            se.wait_ge(s5, t - 1)  # WAR s_n (VE7 read)
                        se.activation(s_n[:, :], s_t2[:, :], af.Tanh).then_inc(s3, 1)
                        continue
                    se.wait_ge(s1, t)
                    if t >= 2:
                        se.wait_ge(d3, t - 1)
                    se.activation(s_rz[:, :], s_g[:, 0:4], af.Sigmoid).then_inc(s2, 1)
                    se.wait_ge(s4, t)
                    if t >= 2:
                        se.wait_ge(s5, t - 1)
                    se.activation(s_n[:, :], s_t2[:, :], af.Tanh).then_inc(s3, 1)
                    se.wait_ge(s5, t)
                    if rd and t >= 3:
                        se.wait_ge(lsem, 16 * (t - 2))  # WAR: send buf slot b
                    se.activation(s_hbf[:, 2 * b : 2 * b + 2], s_hf[:, :], af.Copy).then_inc(s6, 1)
                for tt in range(n_gemm_tiles):
                    ntt = min(512, t_hist - tt * 512)
                    se.wait_ge(s_gemm, tt + 1)
                    se.activation(
                        s_ob[tt][0:OC, 0:ntt], po[tt][0:OC, 0:ntt],
                        af.Identity, bias=s_bc[0:OC, 0:1],
                    ).then_inc(s_out, 1)

            if rd:
                hsem = s5 if exchange == "rdma3" else s6

                @block.gpsimd
                def _(g):
                    g.load_library(library_config.remote_dma)
                    pid2 = g.partition_id() * 2
                    offs = [pid2, pid2 + NK]  # landing col by parity
                    rdests = [(0, k) for k in range(N_CORES)]
                    for t in range(1, S + 1):
                        b = t % 2
                        g.remote_dma_broadcast(
                            out_ap=bass.AP(s_land, offs[b], [[2 * NK, 128], [1, 2]]),
                            in_ap=s_hbf[:, 2 * b : 2 * b + 2],
                            remote_sem=rsem,
                            local_sem=lsem,
                            rdests=rdests,
                        ).then_inc(psem, 1)
                        g.wait_ge(psem, t)
                        g.wait_ge(hsem, t)  # send data ready
                        g.trigger_dma(count=1)

    if rd:
        # Raw Bass skips the extended-inst ISA encode pass; without it the
        # NEFF compiler sees empty .instr -> "ISA wrong length".
        from concourse.library_overlay import lower_extended_insts
        lower_extended_insts(nc)
    return nc


def _host_precompute(inputs):
    """fp64 host precompute: merged gates, collapsed chain, step 0."""
    f64 = lambda k: np.asarray(inputs[k], np.float64)
    W_ih, W_hh = f64("W_ih"), f64("W_hh")
    b_ih, b_hh = f64("b_ih"), f64("b_hh")

    W_rz = W_ih[: 2 * H] + W_hh[: 2 * H]
    b_rz = b_ih[: 2 * H] + b_hh[: 2 * H]
    W_in_, b_in_ = W_ih[2 * H :], b_ih[2 * H :]
    W_hn, b_hn = W_hh[2 * H :], b_hh[2 * H :]

    W_chain = f64("Wo") @ f64("W4") @ f64("W3") @ f64("W2") @ f64("W1")
    b_chain = (
        f64("Wo") @ (f64("W4") @ (f64("W3") @ (f64("W2") @ f64("b1") + f64("b2")) + f64("b3")) + f64("b4"))
        + f64("bo")
    )

    sig = lambda x: 1.0 / (1.0 + np.exp(-x))
    h0 = f64("hidden")[0]
    gi = b_ih
    gh = W_hh @ h0 + b_hh
    r = sig(gi[:H] + gh[:H])
    z = sig(gi[H : 2 * H] + gh[H : 2 * H])
    n = np.tanh(gi[2 * H :] + r * gh[2 * H :])
    h1 = ((1.0 - z) * n + z * h0).astype(np.float32)

    return W_rz, b_rz, W_in_, b_in_, W_hn, b_hn, W_chain, b_chain, h1


def _perm2():
    # hist layout: hist[p, t*NK + kk] = h_t[(kk//2)*256 + (kk%2)*128 + p]
    # i.e. sender c's [128,2] slab lands contiguously at columns 2c..2c+1.
    p = np.arange(128)[:, None]
    kk = np.arange(NK)[None, :]
    return (kk // 2) * 256 + (kk % 2) * 128 + p


PERM2 = _perm2()


def make_in_maps(inputs):
    W_rz, b_rz, W_in_, b_in_, W_hn, b_hn, W_chain, b_chain, h1 = _host_precompute(inputs)
    bf = lambda x: np.ascontiguousarray(x.astype(np.float32).astype(ml_dtypes.bfloat16))
    f32 = lambda x: np.ascontiguousarray(x.astype(np.float32))

    in_maps = []
    for i in range(N_CORES):
        sl = slice(256 * i, 256 * (i + 1))
        rows = np.concatenate([W_rz[:H][sl], W_rz[H:][sl], W_in_[sl], W_hn[sl]], axis=0)  # [1024, 2048]
        wg = rows.reshape(NR, 128, H)[:, :, PERM2].transpose(2, 3, 0, 1).reshape(128, NK * NR * 128)
        wci = W_chain[OC * i : OC * (i + 1)]  # [64, 2048]
        wc = wci[:, PERM2].transpose(1, 2, 0).reshape(128, NK * OC)
        bias_rows = np.concatenate([b_rz[:H][sl], b_rz[H:][sl], b_in_[sl], b_hn[sl]])  # [1024]
        bg = np.ascontiguousarray(bias_rows.reshape(NR, 128).T)  # [128, NR]
        bc = b_chain[OC * i : OC * (i + 1)].reshape(OC, 1)
        h1b = np.ascontiguousarray(h1[PERM2])  # [128, NK]
        h1f = np.ascontiguousarray(h1[sl].reshape(2, 128).T)  # [128, 2]
        in_maps.append({
            "wg": bf(wg), "wc": bf(wc), "bg": f32(bg), "bc": f32(bc),
            "h1b": bf(h1b), "h1f": f32(h1f), "h1fb": bf(h1f),
        })
    return in_maps


def kernel(**inputs):
    global LAST_RESULTS
    from concourse.bass_utils import run_bass_kernel_spmd

    in_maps = make_in_maps(inputs)
    nc = build_nc(T, exchange="rdma3")
    res = run_bass_kernel_spmd(nc, in_maps, core_ids=list(range(N_CORES)))
    LAST_RESULTS = res

    out = np.empty((T, OD), np.float32)
    for i in range(N_CORES):
        out[:, OC * i : OC * (i + 1)] = res.results[i]["out"].T
    return out[:, None, :]


# revision 3
# speedup vs baseline: 3.7420x; 1.0717x over previous
"""Trainium2 Bass kernel for nn_Decoder: 1024-step GRU decoder, H=2048, OD=512.

Same math as baseline (host-merged r/z gates, collapsed 5-linear chain, step 0
on host, bf16 TP-8 gate GEMVs with weights resident in SBUF), but the per-step
h exchange uses remote_dma_broadcast (SBUF->SBUF, all 8 cores, one instruction
per step, landing slot indexed by a sender-id register) instead of
collective_compute AllGather + DRAM bounce DMAs.

Exchange lands in a small double-buffered pad s_land[128, 2*NK] (big-stride APs
don't encode in the remote-DMA descs); the DVE archives each slot into s_hist
off the critical path for the final output GEMM.

exchange modes:
  'rdma'  - remote-DMA exchange, baseline gate chain.
  'rdma3' - rdma + PE r/z-first split (gate activations overlap the in/hn
            matmuls), shortened post-tanh chain (zh = z*h and 1-z precomputed
            under the PE), bf16 h state (DVE writes the send buffer directly;
            no scalar-engine copy on the critical path).
  'none'  - no exchange at all (reads stale slot 0); timing skeleton only.
"""

import os
import sys

import numpy as np
import ml_dtypes

for p in ("/opt/trn_rl_repo", "/root/.axon_site/_ro/trn_rl_repo"):
    if os.path.isdir(p) and p not in sys.path:
        sys.path.append(p)

import concourse.bass as bass
import concourse.mybir as mybir
from concourse import library_config

H = 2048
OD = 512
T = 1024
N_CORES = 8
NK = 16  # contraction chunks of 128
NR = 8   # gate row tiles of 128 per core
ROWS = NR * 128  # 1024 gate rows per core
OC = OD // N_CORES  # 64 output cols per core

BF16 = mybir.dt.bfloat16
F32 = mybir.dt.float32

LAST_RESULTS = None


def build_nc(t_hist=T, exchange="rdma3"):
    """Build the SPMD program. t_hist = number of h-history slots (incl. slot 0);
    device runs S = t_hist-1 recurrence steps."""
    S = t_hist - 1
    af = mybir.ActivationFunctionType

    nc = bass.Bass("TRN2", target_bir_lowering=False, debug=False,
                   num_devices=N_CORES)

    wg_d = nc.declare_dram_parameter("wg", [128, NK * NR * 128], BF16, isOutput=False)
    wc_d = nc.declare_dram_parameter("wc", [128, NK * OC], BF16, isOutput=False)
    bg_d = nc.declare_dram_parameter("bg", [128, NR], F32, isOutput=False)
    bc_d = nc.declare_dram_parameter("bc", [OC, 1], F32, isOutput=False)
    h1b_d = nc.declare_dram_parameter("h1b", [128, NK], BF16, isOutput=False)
    h1f_d = nc.declare_dram_parameter("h1f", [128, 2], F32, isOutput=False)
    h1fb_d = nc.declare_dram_parameter("h1fb", [128, 2], BF16, isOutput=False)
    out_d = nc.declare_dram_parameter("out", [OC, t_hist], F32, isOutput=True)

    n_gemm_tiles = (t_hist + 511) // 512

    from contextlib import ExitStack
    with ExitStack() as ctx:
        E = ctx.enter_context
        s_wg = E(nc.sbuf_tensor("s_wg", [128, NK * NR * 128], BF16))
        s_wc = E(nc.sbuf_tensor("s_wc", [128, NK * OC], BF16))
        s_bg = E(nc.sbuf_tensor("s_bg", [128, NR], F32))
        s_bc = E(nc.sbuf_tensor("s_bc", [OC, 1], F32))
        s_hist = E(nc.sbuf_tensor("s_hist", [128, NK * t_hist], BF16))
        s_land = E(nc.sbuf_tensor("s_land", [128, 2 * NK], BF16))  # 2 parities
        s_g = E(nc.sbuf_tensor("s_g", [128, NR], F32))
        s_rz = E(nc.sbuf_tensor("s_rz", [128, 4], F32))
        s_t1 = E(nc.sbuf_tensor("s_t1", [128, 2], F32))
        s_t2 = E(nc.sbuf_tensor("s_t2", [128, 2], F32))
        s_n = E(nc.sbuf_tensor("s_n", [128, 2], F32))
        s_d = E(nc.sbuf_tensor("s_d", [128, 2], F32))
        s_zd = E(nc.sbuf_tensor("s_zd", [128, 2], F32))
        s_hf = E(nc.sbuf_tensor("s_hf", [128, 2], F32))
        s_hbf = E(nc.sbuf_tensor("s_hbf", [128, 4], BF16))  # send buf, 2 slots
        s_grz = E(nc.sbuf_tensor("s_grz", [128, 4], F32))
        s_gin = E(nc.sbuf_tensor("s_gin", [128, 4], F32))
        s_zh = E(nc.sbuf_tensor("s_zh", [128, 2], F32))
        s_omz = E(nc.sbuf_tensor("s_omz", [128, 2], F32))
        s_c = E(nc.sbuf_tensor("s_c", [128, 2], F32))
        s_ob = [E(nc.sbuf_tensor(f"s_ob{i}", [OC, 512], F32)) for i in range(n_gemm_tiles)]

        pg = [E(nc.psum_tensor(f"pg{i}", [128, 512], F32)) for i in range(2)]
        pgb = [E(nc.psum_tensor(f"pgb{i}", [128, 512], F32)) for i in range(2)]
        po = [E(nc.psum_tensor(f"po{i}", [128, 512], F32)) for i in range(n_gemm_tiles)]
        n_dummy = int(os.environ.get("N_DUMMY", "0"))

        dma_in = E(nc.semaphore("dma_in"))
        dma_out = E(nc.semaphore("dma_out"))
        s_pe = E(nc.semaphore("s_pe"))
        s_peb = E(nc.semaphore("s_peb"))
        s_peb2 = E(nc.semaphore("s_peb2"))
        s1 = E(nc.semaphore("s1"))
        s2 = E(nc.semaphore("s2"))
        s3 = E(nc.semaphore("s3"))
        s4 = E(nc.semaphore("s4"))
        s5 = E(nc.semaphore("s5"))
        s6 = E(nc.semaphore("s6"))
        s_gemm = E(nc.semaphore("s_gemm"))
        s_out = E(nc.semaphore("s_out"))
        d1 = E(nc.semaphore("d1"))
        d2 = E(nc.semaphore("d2"))
        d3 = E(nc.semaphore("d3"))
        rd = exchange in ("rdma", "rdma3")
        if rd:
            rsem = E(nc.semaphore("rsem"))   # +2 per arriving slice; 16/step
            lsem = E(nc.semaphore("lsem"))   # +16 per completed local send
            psem = E(nc.semaphore("psem"))   # +1 per desc-gen prep
            csem = E(nc.semaphore("csem"))   # +1 per archived hist slot

        NIN = 128  # 8 initial DMAs x 16

        with nc.Block() as block:

            @block.sync
            def _(sp):
                sp.dma_start(out=s_wg[:, :], in_=wg_d[:, :]).then_inc(dma_in, 16)
                sp.dma_start(out=s_wc[:, :], in_=wc_d[:, :]).then_inc(dma_in, 16)
                sp.dma_start(out=s_bg[:, :], in_=bg_d[:, :]).then_inc(dma_in, 16)
                sp.dma_start(out=s_bc[:, :], in_=bc_d[:, :]).then_inc(dma_in, 16)
                sp.dma_start(out=s_hist[:, 0:NK], in_=h1b_d[:, :]).then_inc(dma_in, 16)
                sp.dma_start(out=s_land[:, 0:NK], in_=h1b_d[:, :]).then_inc(dma_in, 16)
                sp.dma_start(out=s_hf[:, :], in_=h1f_d[:, :]).then_inc(dma_in, 16)
                sp.dma_start(out=s_hbf[:, 0:2], in_=h1fb_d[:, :]).then_inc(dma_in, 16)
                for tt in range(n_gemm_tiles):
                    ntt = min(512, t_hist - tt * 512)
                    sp.wait_ge(s_out, tt + 1)
                    sp.dma_start(
                        out=out_d[0:OC, tt * 512 : tt * 512 + ntt],
                        in_=s_ob[tt][0:OC, 0:ntt],
                    ).then_inc(dma_out, 16)
                sp.wait_ge(dma_out, 16 * n_gemm_tiles)

            @block.tensor
            def _(te):
                te.wait_ge(dma_in, NIN)
                for t in range(1, S + 1):
                    if rd:
                        if t >= 2:
                            te.wait_ge(rsem, 16 * (t - 1))
                        rhs_base = NK * ((t - 1) % 2)
                    else:
                        if t >= 2:
                            te.wait_ge(s6, t - 1)
                        rhs_base = 0
                    p = pg[t % 2]
                    if exchange == "rdma3":
                        # r/z tiles (rt 0-3) first so gate activations overlap
                        # the in/hn tiles. rt-outer (groups must not interleave
                        # within a bank); separate banks per group so the DVE
                        # can read group A while group B still accumulates.
                        # r/z tiles (pg cols 0:4), then h_n (pgb cols 2:4),
                        # then i_n (pg cols 4:6). The s1 wait proves the DVE's
                        # r/z read retired before the pg bank re-opens; h_n
                        # lands first so bias+r*h_n overlap the i_n tiles.
                        pb = pgb[t % 2]
                        for rt in (0, 1, 2, 3, 6, 7, 4, 5):
                            if rt < 4:
                                dst = p[:, rt : rt + 1]
                            elif rt >= 6:
                                dst = pb[:, rt - 4 : rt - 3]
                            else:
                                dst = p[:, rt : rt + 1]  # i_n in pg cols 4:6
                            if rt == 4:
                                te.wait_ge(s1, t)  # DVE done reading pg 0:4
                            for c in range(NK):
                                mm = te.matmul(
                                    dst,
                                    lhsT=s_wg[:, (c * NR + rt) * 128 : (c * NR + rt + 1) * 128],
                                    rhs=s_land[:, rhs_base + c : rhs_base + c + 1],
                                    start=(c == 0),
                                    stop=(c == NK - 1),
                                )
                            if rt == 3:
                                mm.then_inc(s_pe, 1)
                            elif rt == 7:
                                mm.then_inc(s_peb, 1)
                        mm.then_inc(s_peb2, 1)
                        # keep the PE clocked up through the exchange window
                        # (po[0] gets start=True-reset by the final GEMM later)
                        for w in range(n_dummy if t < S else 0):
                            te.matmul(
                                po[0][:, 0:1],
                                lhsT=s_wg[:, (w % 128) * 128 : (w % 128) * 128 + 128],
                                rhs=s_land[:, rhs_base : rhs_base + 1],
                                start=True,
                                stop=True,
                            )
                    else:
                        for rt in range(NR):
                            for c in range(NK):
                                mm = te.matmul(
                                    p[:, rt : rt + 1],
                                    lhsT=s_wg[:, (c * NR + rt) * 128 : (c * NR + rt + 1) * 128],
                                    rhs=s_land[:, rhs_base + c : rhs_base + c + 1],
                                    start=(c == 0),
                                    stop=(c == NK - 1),
                                )
                        mm.then_inc(s_pe, 1)
                # final GEMM: out[m, tau] = sum_k Wc[m,k] hist[tau,k]
                if rd and S >= 1:
                    te.wait_ge(csem, S)  # all slots archived
                for tt in range(n_gemm_tiles):
                    ntt = min(512, t_hist - tt * 512)
                    for c in range(NK):
                        mm = te.matmul(
                            po[tt][0:OC, 0:ntt],
                            lhsT=s_wc[:, c * OC : (c + 1) * OC],
                            rhs=bass.AP(s_hist, tt * 512 * NK + c, [[NK * t_hist, 128], [NK, ntt]]),
                            start=(c == 0),
                            stop=(c == NK - 1),
                        )
                    mm.then_inc(s_gemm, 1)

            @block.vector
            def _(ve):
                ve.wait_ge(dma_in, NIN)
                for t in range(1, S + 1):
                    b = t % 2
                    # archive slot t-1 (parity (t-1)%2) into s_hist; off the
                    # critical path. Safe vs bcast t+1 (same parity): senders
                    # gate on our step-t gates, which follow this in VE order.
                    if rd and t >= 2:
                        ve.wait_ge(rsem, 16 * (t - 1))
                        ve.tensor_scalar_add(
                            s_hist[:, (t - 1) * NK : t * NK],
                            s_land[:, NK * ((t - 1) % 2) : NK * ((t - 1) % 2) + NK],
                            0.0,
                        ).then_inc(csem, 1)
                    p = pg[t % 2]
                    if exchange == "rdma3":
                        ve.wait_ge(s_pe, t)
                        if t >= 2:
                            ve.wait_ge(s2, t - 1)  # WAR s_grz (sigmoid read)
                        ve.tensor_add(s_grz[:, :], p[:, 0:4], s_bg[:, 0:4]).then_inc(s1, 1)
                        ve.wait_ge(s2, t)
                        if t >= 2:
                            ve.wait_ge(s5, t - 1)  # V8(t-1) retired: s_zh/s_omz WAR
                            ve.wait_ge(s6, t - 1)  # f32 master h updated (V9 t-1)
                        # under PE group B: omz = 1-z, zh = z*h (f32 master)
                        ve.tensor_scalar(
                            s_omz[:, :], s_rz[:, 2:4], -1.0, 1.0,
                            mybir.AluOpType.mult, mybir.AluOpType.add,
                        )
                        ve.tensor_mul(s_zh[:, :], s_rz[:, 2:4], s_hf[:, :])
                        ve.wait_ge(s_peb, t)  # h_n tiles done (pgb closed)
                        if t >= 2:
                            ve.wait_ge(s4, t - 1)  # WAR s_gin/s_t1 (V5/V6 t-1)
                        ve.tensor_add(s_gin[:, 2:4], pgb[t % 2][:, 2:4], s_bg[:, 6:8]).then_inc(d1, 1)
                        ve.wait_ge(d1, 2 * t - 1)
                        ve.tensor_mul(s_t1[:, :], s_rz[:, 0:2], s_gin[:, 2:4]).then_inc(d2, 1)
                        ve.wait_ge(s_peb2, t)  # i_n tiles done (pg closed)
                        ve.tensor_add(s_gin[:, 0:2], pg[t % 2][:, 4:6], s_bg[:, 4:6]).then_inc(d1, 1)
                        ve.wait_ge(d1, 2 * t)
                        ve.wait_ge(d2, t)
                        if t >= 2:
                            ve.wait_ge(s3, t - 1)  # WAR s_t2 (tanh read)
                        ve.tensor_add(s_t2[:, :], s_t1[:, :], s_gin[:, 0:2]).then_inc(s4, 1)
                        ve.wait_ge(s3, t)
                        ve.tensor_mul(s_c[:, :], s_omz[:, :], s_n[:, :]).then_inc(d3, 1)
                        ve.wait_ge(d3, t)
                        if t >= 3:
                            ve.wait_ge(lsem, 16 * (t - 2))  # WAR send slot b
                        ve.tensor_add(s_hbf[:, 2 * b : 2 * b + 2], s_c[:, :], s_zh[:, :]).then_inc(s5, 1)
                        # f32 master h, off the critical path (read by V2 t+1)
                        ve.tensor_add(s_hf[:, :], s_c[:, :], s_zh[:, :]).then_inc(s6, 1)
                    else:
                        ve.wait_ge(s_pe, t)
                        if t >= 2:
                            ve.wait_ge(d1, t - 1)
                            ve.wait_ge(s4, t - 1)
                        ve.tensor_add(s_g[:, :], p[:, 0:NR], s_bg[:, :]).then_inc(s1, 1)
                        ve.wait_ge(s2, t)
                        ve.tensor_mul(s_t1[:, :], s_rz[:, 0:2], s_g[:, 6:8]).then_inc(d1, 1)
                        ve.wait_ge(d1, t)
                        ve.tensor_add(s_t2[:, :], s_t1[:, :], s_g[:, 4:6]).then_inc(s4, 1)
                        ve.wait_ge(s3, t)
                        if t >= 2:
                            ve.wait_ge(s5, t - 1)
                        ve.tensor_sub(s_d[:, :], s_hf[:, :], s_n[:, :]).then_inc(d2, 1)
                        ve.wait_ge(d2, t)
                        ve.tensor_mul(s_zd[:, :], s_d[:, :], s_rz[:, 2:4]).then_inc(d3, 1)
                        ve.wait_ge(d3, t)
                        ve.tensor_add(s_hf[:, :], s_n[:, :], s_zd[:, :]).then_inc(s5, 1)
                if rd and S >= 1:
                    ve.wait_ge(rsem, 16 * S)
                    ve.tensor_scalar_add(
                        s_hist[:, S * NK : (S + 1) * NK],
                        s_land[:, NK * (S % 2) : NK * (S % 2) + NK],
                        0.0,
                    ).then_inc(csem, 1)

            @block.scalar
            def _(se):
                se.wait_ge(dma_in, NIN)
                for t in range(1, S + 1):
                    b = t % 2
                    if exchange == "rdma3":
                        se.wait_ge(s1, t)
                        if t >= 2:
                            se.wait_ge(s4, t - 1)  # WAR s_rz (VE reads)
                        se.activation(s_rz[:, :], s_grz[:, :], af.Sigmoid).then_inc(s2, 1)
                        se.wait_ge(s4, t)
                        if t >= 2:
                            se.wait_ge(s5, t - 1)  # WAR s_n (VE7 read)
                        se.activation(s_n[:, :], s_t2[:, :], af.Tanh).then_inc(s3, 1)
                        continue
                    se.wait_ge(s1, t)
                    if t >= 2:
                        se.wait_ge(d3, t - 1)
                    se.activation(s_rz[:, :], s_g[:, 0:4], af.Sigmoid).then_inc(s2, 1)
                    se.wait_ge(s4, t)
                    if t >= 2:
                        se.wait_ge(s5, t - 1)
                    se.activation(s_n[:, :], s_t2[:, :], af.Tanh).then_inc(s3, 1)
                    se.wait_ge(s5, t)
                    if rd and t >= 3:
                        se.wait_ge(lsem, 16 * (t - 2))  # WAR: send buf slot b
                    se.activation(s_hbf[:, 2 * b : 2 * b + 2], s_hf[:, :], af.Copy).then_inc(s6, 1)
                for tt in range(n_gemm_tiles):
                    ntt = min(512, t_hist - tt * 512)
                    se.wait_ge(s_gemm, tt + 1)
                    se.activation(
                        s_ob[tt][0:OC, 0:ntt], po[tt][0:OC, 0:ntt],
                        af.Identity, bias=s_bc[0:OC, 0:1],
                    ).then_inc(s_out, 1)

            if rd:
                hsem = s5 if exchange == "rdma3" else s6

                @block.gpsimd
                def _(g):
                    g.load_library(library_config.remote_dma)
                    pid2 = g.partition_id() * 2
                    offs = [pid2, pid2 + NK]  # landing col by parity
                    rdests = [(0, k) for k in range(N_CORES)]
                    for t in range(1, S + 1):
                        b = t % 2
                        g.remote_dma_broadcast(
                            out_ap=bass.AP(s_land, offs[b], [[2 * NK, 128], [1, 2]]),
                            in_ap=s_hbf[:, 2 * b : 2 * b + 2],
                            remote_sem=rsem,
                            local_sem=lsem,
                            rdests=rdests,
                        ).then_inc(psem, 1)
                        g.wait_ge(psem, t)
                        g.wait_ge(hsem, t)  # send data ready
                        g.trigger_dma(count=1)

    if rd:
        # Raw Bass skips the extended-inst ISA encode pass; without it the
        # NEFF compiler sees empty .instr -> "ISA wrong length".
        from concourse.library_overlay import lower_extended_insts
        lower_extended_insts(nc)
    return nc


def _host_precompute(inputs):
    """fp64 host precompute: merged gates, collapsed chain, step 0."""
    f64 = lambda k: np.asarray(inputs[k], np.float64)
    W_ih, W_hh = f64("W_ih"), f64("W_hh")
    b_ih, b_hh = f64("b_ih"), f64("b_hh")

    W_rz = W_ih[: 2 * H] + W_hh[: 2 * H]
    b_rz = b_ih[: 2 * H] + b_hh[: 2 * H]
    W_in_, b_in_ = W_ih[2 * H :], b_ih[2 * H :]
    W_hn, b_hn = W_hh[2 * H :], b_hh[2 * H :]

    W_chain = f64("Wo") @ f64("W4") @ f64("W3") @ f64("W2") @ f64("W1")
    b_chain = (
        f64("Wo") @ (f64("W4") @ (f64("W3") @ (f64("W2") @ f64("b1") + f64("b2")) + f64("b3")) + f64("b4"))
        + f64("bo")
    )

    sig = lambda x: 1.0 / (1.0 + np.exp(-x))
    h0 = f64("hidden")[0]
    gi = b_ih
    gh = W_hh @ h0 + b_hh
    r = sig(gi[:H] + gh[:H])
    z = sig(gi[H : 2 * H] + gh[H : 2 * H])
    n = np.tanh(gi[2 * H :] + r * gh[2 * H :])
    h1 = ((1.0 - z) * n + z * h0).astype(np.float32)

    return W_rz, b_rz, W_in_, b_in_, W_hn, b_hn, W_chain, b_chain, h1


def _perm2():
    # hist layout: hist[p, t*NK + kk] = h_t[(kk//2)*256 + (kk%2)*128 + p]
    # i.e. sender c's [128,2] slab lands contiguously at columns 2c..2c+1.
    p = np.arange(128)[:, None]
    kk = np.arange(NK)[None, :]
    return (kk // 2) * 256 + (kk % 2) * 128 + p


PERM2 = _perm2()


def make_in_maps(inputs):
    W_rz, b_rz, W_in_, b_in_, W_hn, b_hn, W_chain, b_chain, h1 = _host_precompute(inputs)
    bf = lambda x: np.ascontiguousarray(x.astype(np.float32).astype(ml_dtypes.bfloat16))
    f32 = lambda x: np.ascontiguousarray(x.astype(np.float32))

    in_maps = []
    for i in range(N_CORES):
        sl = slice(256 * i, 256 * (i + 1))
        rows = np.concatenate([W_rz[:H][sl], W_rz[H:][sl], W_in_[sl], W_hn[sl]], axis=0)  # [1024, 2048]
        wg = rows.reshape(NR, 128, H)[:, :, PERM2].transpose(2, 3, 0, 1).reshape(128, NK * NR * 128)
        wci = W_chain[OC * i : OC * (i + 1)]  # [64, 2048]
        wc = wci[:, PERM2].transpose(1, 2, 0).reshape(128, NK * OC)
        bias_rows = np.concatenate([b_rz[:H][sl], b_rz[H:][sl], b_in_[sl], b_hn[sl]])  # [1024]
        bg = np.ascontiguousarray(bias_rows.reshape(NR, 128).T)  # [128, NR]
        bc = b_chain[OC * i : OC * (i + 1)].reshape(OC, 1)
        h1b = np.ascontiguousarray(h1[PERM2])  # [128, NK]
        h1f = np.ascontiguousarray(h1[sl].reshape(2, 128).T)  # [128, 2]
        in_maps.append({
            "wg": bf(wg), "wc": bf(wc), "bg": f32(bg), "bc": f32(bc),
            "h1b": bf(h1b), "h1f": f32(h1f), "h1fb": bf(h1f),
        })
    return in_maps


def kernel(**inputs):
    global LAST_RESULTS
    from concourse.bass_utils import run_bass_kernel_spmd

    in_maps = make_in_maps(inputs)
    nc = build_nc(T, exchange="rdma3")
    res = run_bass_kernel_spmd(nc, in_maps, core_ids=list(range(N_CORES)))
    LAST_RESULTS = res

    out = np.empty((T, OD), np.float32)
    for i in range(N_CORES):
        out[:, OC * i : OC * (i + 1)] = res.results[i]["out"].T
    return out[:, None, :]
